# revision 14
# baseline (speedup 1.0000x reference)
"""CrossModalMatchingNetwork Trainium2 kernel.

Full-input contract: kernel(**inputs) takes the unsharded numpy inputs and
returns the full [B, S, S] cosine-similarity output (float32).

Strategy: data-parallel over batch across 8 NeuronCores (2 batches/core).
Host-side prep transposes the big activations to [D, S] layout so the
contraction dim lands on SBUF partitions, casts them to bf16 (fp32 PSUM
accumulation keeps the error ~5e-3), packs the projection weights into a
partition-major [P, K, H] layout (contiguous per partition, so weight DMAs
are ~128 fat descriptors instead of ~2000 1KB ones), and replicates the
weights to every core.

Per core, per batch (n2 indexes the two 512-column halves of S):
  projT: tT[h, s]  = sum_d WtT[d,h] * txtT[d,s] + bt[h]
  T-norm chain per n2 (overlaps the other half's matmuls):
     tsq = tT^2 -> tss = sum_m tsq -> ones-matmul (sum over partitions)
     -> sqrt row -> replicate via K=1 matmul -> reciprocal (DVE)
     -> fold 1/tn into tT (so the dots epilogue is a pure PSUM copy)
  projV + V-norm chain likewise folds 1/vn into vT.
  dots i-loop: psum = vT_i^T @ tT -> copy to SBUF (scalar/vector split)
  -> DMA out.

Batch 0 is latency-critical (nothing to overlap with the input DMAs), so
its inputs stream in per-k chunks in priority order (txt+wt first, then
wv+vis interleaved) and the first projection half runs k-OUTER across four
concurrent PSUM accumulation groups, consuming each chunk as it lands —
the PE starts ~8.5us in instead of waiting ~18us for full tensors.
Doorbells are spread across engine queues (wt on scalar, txt/vis/wv on
sync, consts on gpsimd) so issue ops never sit in front of compute ops.
"""

import numpy as np
from contextlib import ExitStack

import concourse.bass as bass
import concourse.mybir as mybir
import concourse.tile as tile
from concourse import bacc
from concourse.bass import ds, ts

B, S, VD, TD, H = 16, 1024, 1024, 768, 512
NCORES = 8
BPC = B // NCORES  # batches per core
P = 128
FD = 512  # matmul moving-operand free dim (one PSUM bank of fp32)

F32 = mybir.dt.float32
F32R = mybir.dt.float32r
BF16 = mybir.dt.bfloat16

AF = mybir.ActivationFunctionType

N_WARMUP = 6


def build(bpc=BPC, s=S, vd=VD, td=TD, h=H, dtype="bf16"):
    fd = min(FD, s)
    kv, kt, mh = vd // P, td // P, h // P
    ns, ms = s // fd, s // P

    if dtype == "bf16":
        CT = BF16
        _w = lambda ap: ap  # noqa: E731
    else:
        CT = F32
        _w = lambda ap: ap.bitcast(F32R)  # noqa: E731  # fp32r: single-pass PE

    nc = bacc.Bacc("TRN2", target_bir_lowering=False)
    visT = nc.dram_tensor("visT", [bpc, vd, s], CT, kind="ExternalInput")
    txtT = nc.dram_tensor("txtT", [bpc, td, s], CT, kind="ExternalInput")
    wvp = nc.dram_tensor("wvp", [P, kv, h], CT, kind="ExternalInput")
    wtp = nc.dram_tensor("wtp", [P, kt, h], CT, kind="ExternalInput")
    bvp = nc.dram_tensor("bvp", [P, mh], F32, kind="ExternalInput")
    btp = nc.dram_tensor("btp", [P, mh], F32, kind="ExternalInput")
    onesd = nc.dram_tensor("ones", [P, P], CT, kind="ExternalInput")
    out = nc.dram_tensor("out", [bpc, s, s], F32, kind="ExternalOutput")

    with (
        tile.TileContext(nc) as tc,
        ExitStack() as ctx,
        nc.allow_low_precision(reason="compute dtype is bf16 by design"),
    ):
        consts = ctx.enter_context(tc.tile_pool(name="consts", bufs=1))
        vis_pool = ctx.enter_context(tc.tile_pool(name="vis", bufs=2))
        txt_pool = ctx.enter_context(tc.tile_pool(name="txt", bufs=2))
        proj_pool = ctx.enter_context(tc.tile_pool(name="proj", bufs=1))
        sq_pool = ctx.enter_context(tc.tile_pool(name="sq", bufs=2))
        row_pool = ctx.enter_context(tc.tile_pool(name="rows", bufs=4))
        out_pool = ctx.enter_context(tc.tile_pool(name="outs", bufs=3))
        ps_mm = ctx.enter_context(tc.tile_pool(name="ps_mm", bufs=4, space="PSUM"))
        ps_row = ctx.enter_context(tc.tile_pool(name="ps_row", bufs=2, space="PSUM"))
        ps_repl = ctx.enter_context(tc.tile_pool(name="ps_repl", bufs=2, space="PSUM"))

        # --- consts: tiny ones on the gpsimd queue (idle until dots), per-k
        # weight chunks on scalar (wt) / sync (wv, interleaved with vis below)
        bt_sb = consts.tile([P, mh], F32)
        nc.gpsimd.dma_start(bt_sb[:], btp[:, :])
        bv_sb = consts.tile([P, mh], F32)
        nc.gpsimd.dma_start(bv_sb[:], bvp[:, :])
        ones_sb = consts.tile([P, P], CT)
        nc.gpsimd.dma_start(_w(ones_sb[:]), _w(onesd[:, :]))
        wt_sb = consts.tile([P, kt, h], CT)
        for k in range(kt):
            nc.scalar.dma_start(_w(wt_sb[:, k, :]), _w(wtp[:, k, :]))
        wv_sb = consts.tile([P, kv, h], CT)
        ones_col = ones_sb[:, 0:1]
        ones_row = ones_sb[0:1, :]

        # txt b0 chunks first: projT is the critical path at the head
        txt0_sb = txt_pool.tile([P, kt, s], CT, tag="txt")
        for k in range(kt):
            nc.sync.dma_start(_w(txt0_sb[:, k, :]), _w(txtT[0, ds(k * P, P), :]))

        # PE warm-up on a memset tile: just enough to start the clock ramp
        # before the first real chunk lands.
        warm_sb = consts.tile([P, fd], CT)
        nc.vector.memset(warm_sb[:], 0.0)
        warm_ps = ps_repl.tile([P, fd], F32, tag="ps_repl")
        for _ in range(N_WARMUP):
            nc.tensor.matmul(warm_ps[:], _w(warm_sb[:, 0:P]), _w(warm_sb[:]))
        nc.scalar.activation(_w(warm_sb[:, 0:P]), warm_ps[:, 0:P], AF.Copy)

        def proj_group(n2, m, kk, w_sb, b_sb, x_sb, y_sb):
            """y[:, m, n2-half] = W[:, :, m-slice].T @ x + b (one PSUM group)."""
            pv = ps_mm.tile([P, fd], F32, tag="ps_mm")
            for k in range(kk):
                nc.tensor.matmul(
                    pv[:],
                    _w(w_sb[:, k, ts(m, P)]),
                    _w(x_sb[:, k, ds(n2 * fd, fd)]),
                    start=(k == 0),
                    stop=(k == kk - 1),
                )
            nc.scalar.activation(
                _w(y_sb[:, m, ds(n2 * fd, fd)]), pv[:], AF.Identity,
                bias=b_sb[:, ds(m, 1)],
            )

        def proj_kouter(n2, kk, w_sb, b_sb, x_sb, y_sb):
            """All mh groups at once, k outermost: consumes input chunk k as
            soon as its DMA lands (batch-0 head latency)."""
            pvs = [
                ps_mm.tile([P, fd], F32, tag="ps_mm", name=f"pko{m}")
                for m in range(mh)
            ]
            for k in range(kk):
                for m in range(mh):
                    nc.tensor.matmul(
                        pvs[m][:],
                        _w(w_sb[:, k, ts(m, P)]),
                        _w(x_sb[:, k, ds(n2 * fd, fd)]),
                        start=(k == 0),
                        stop=(k == kk - 1),
                    )
            for m in range(mh):
                nc.scalar.activation(
                    _w(y_sb[:, m, ds(n2 * fd, fd)]), pvs[m][:], AF.Identity,
                    bias=b_sb[:, ds(m, 1)],
                )

        def chain(y_sb, n2, tag):
            """Fold 1/sqrt(sum_h y^2) into y's n2 half (columns n2*fd..+fd)."""
            sl = ds(n2 * fd, fd)
            sq = sq_pool.tile([P, mh, fd], CT, tag=f"sq{tag}{n2}")
            nc.vector.tensor_mul(_w(sq[:]), y_sb[:, :, sl], y_sb[:, :, sl])
            ss = sq_pool.tile([P, fd], CT, tag=f"ss{tag}{n2}")
            nc.vector.tensor_add(_w(ss[:]), sq[:, 0, :], sq[:, 1, :])
            for m in range(2, mh):
                nc.vector.tensor_add(_w(ss[:]), ss[:], sq[:, m, :])
            pn = ps_row.tile([1, fd], F32, tag="ps_row")
            nc.tensor.matmul(pn[:], _w(ones_col), _w(ss[:]))
            nrow = row_pool.tile([1, fd], CT, tag=f"n{tag}{n2}")
            nc.scalar.activation(_w(nrow[:]), pn[:], AF.Sqrt)
            rp = ps_repl.tile([P, fd], F32, tag="ps_repl")
            nc.tensor.matmul(rp[:], _w(ones_row), _w(nrow[:]))
            rbc = row_pool.tile([P, fd], F32, tag=f"r{tag}{n2}")
            nc.vector.reciprocal_approx_fast(out=rbc[:], in_=rp[:])
            for m in range(mh):
                nc.vector.tensor_mul(
                    _w(y_sb[:, m, sl]), y_sb[:, m, sl], rbc[:]
                )

        def dots_tile(b, i, vt_sb, tt_sb, last=False):
            pds = []
            for jc in range(ns):
                pd = ps_mm.tile([P, fd], F32, tag="ps_mm")
                for hc in range(mh):
                    nc.tensor.matmul(
                        pd[:],
                        _w(vt_sb[:, hc, ts(i, P)]),
                        _w(tt_sb[:, hc, ds(jc * fd, fd)]),
                        start=(hc == 0),
                        stop=(hc == mh - 1),
                    )
                pds.append(pd)
            out_sb = out_pool.tile([P, s], F32)
            # epilogue copies split across scalar/vector to balance engines
            nc.scalar.activation(out_sb[:, ds(0, fd)], pds[0][:], AF.Copy)
            if last:
                nc.gpsimd.dma_start(out[b, ts(i, P), ds(0, fd)], out_sb[:, ds(0, fd)])
            nc.vector.tensor_scalar_mul(out_sb[:, ds(fd, fd)], pds[1][:], 1.0)
            if last:
                nc.gpsimd.dma_start(out[b, ts(i, P), ds(fd, fd)], out_sb[:, ds(fd, fd)])
            else:
                nc.gpsimd.dma_start(out[b, ts(i, P), :], out_sb[:])

        # ---------------- batch 0: chunk-paced head ----------------
        vis0_sb = vis_pool.tile([P, kv, s], CT, tag="vis")
        vt_sb = proj_pool.tile([P, mh, s], CT, tag="vt")
        tt_sb = proj_pool.tile([P, mh, s], CT, tag="tt")

        proj_kouter(0, kt, wt_sb, bt_sb, txt0_sb, tt_sb)
        for m in range(mh):
            proj_group(1, m, kt, wt_sb, bt_sb, txt0_sb, tt_sb)
        chain(tt_sb, 0, "t")

        # wv + vis b0 chunks, interleaved so projV's k-outer pass streams;
        # then the (whole-tensor) b1 loads queue up behind them.
        for k in range(kv):
            nc.sync.dma_start(_w(wv_sb[:, k, :]), _w(wvp[:, k, :]))
            nc.sync.dma_start(_w(vis0_sb[:, k, :]), _w(visT[0, ds(k * P, P), :]))
        txt1_sb = txt_pool.tile([P, kt, s], CT, tag="txt")
        nc.sync.dma_start(
            _w(txt1_sb[:]), _w(txtT[1, :, :].rearrange("(k p) s -> p k s", p=P))
        )
        vis1_sb = vis_pool.tile([P, kv, s], CT, tag="vis")
        nc.sync.dma_start(
            _w(vis1_sb[:]), _w(visT[1, :, :].rearrange("(k p) s -> p k s", p=P))
        )

        proj_kouter(0, kv, wv_sb, bv_sb, vis0_sb, vt_sb)
        chain(tt_sb, 1, "t")
        for m in range(mh - 1):
            proj_group(1, m, kv, wv_sb, bv_sb, vis0_sb, vt_sb)
        chain(vt_sb, 0, "v")
        proj_group(1, mh - 1, kv, wv_sb, bv_sb, vis0_sb, vt_sb)

        dots_tile(0, 0, vt_sb, tt_sb)
        dots_tile(0, 1, vt_sb, tt_sb)
        chain(vt_sb, 1, "v")
        for i in range(2, ms):
            dots_tile(0, i, vt_sb, tt_sb)

        # ---------------- batch 1: everything resident ----------------
        vt1_sb = proj_pool.tile([P, mh, s], CT, tag="vt")
        tt1_sb = proj_pool.tile([P, mh, s], CT, tag="tt")
        for m in range(mh):
            proj_group(0, m, kt, wt_sb, bt_sb, txt1_sb, tt1_sb)
        for m in range(mh):
            proj_group(1, m, kt, wt_sb, bt_sb, txt1_sb, tt1_sb)
        chain(tt1_sb, 0, "t")
        for m in range(mh):
            proj_group(0, m, kv, wv_sb, bv_sb, vis1_sb, vt1_sb)
        chain(tt1_sb, 1, "t")
        for m in range(mh - 1):
            proj_group(1, m, kv, wv_sb, bv_sb, vis1_sb, vt1_sb)
        chain(vt1_sb, 0, "v")
        proj_group(1, mh - 1, kv, wv_sb, bv_sb, vis1_sb, vt1_sb)

        dots_tile(1, 0, vt1_sb, tt1_sb)
        dots_tile(1, 1, vt1_sb, tt1_sb)
        chain(vt1_sb, 1, "v")
        for i in range(2, ms):
            dots_tile(1, i, vt1_sb, tt1_sb, last=(i == ms - 1))

    nc.compile()
    return nc


_CACHE = {}


def _get_nc(dtype="bf16"):
    if dtype not in _CACHE:
        _CACHE[dtype] = build(dtype=dtype)
    return _CACHE[dtype]


def _prep_in_maps(visual_features, text_features, Wv, bv, Wt, bt, dtype="bf16"):
    import ml_dtypes

    f = np.float32
    ct = ml_dtypes.bfloat16 if dtype == "bf16" else f
    kv, kt, mh = VD // P, TD // P, H // P
    # packed weights: [P, K, H], partition-major so each partition's slab of
    # K*H*2 bytes is contiguous in DRAM
    wvp = np.ascontiguousarray(
        np.asarray(Wv, dtype=f).T.reshape(kv, P, H).transpose(1, 0, 2)
    ).astype(ct)
    wtp = np.ascontiguousarray(
        np.asarray(Wt, dtype=f).T.reshape(kt, P, H).transpose(1, 0, 2)
    ).astype(ct)
    bvp = np.ascontiguousarray(np.asarray(bv, dtype=f).reshape(mh, P).T)
    btp = np.ascontiguousarray(np.asarray(bt, dtype=f).reshape(mh, P).T)
    ones = np.ones((P, P), dtype=np.float32).astype(ct)
    vis = np.asarray(visual_features, dtype=f)
    txt = np.asarray(text_features, dtype=f)
    in_maps = []
    for c in range(NCORES):
        sl = slice(c * BPC, (c + 1) * BPC)
        in_maps.append({
            "visT": np.ascontiguousarray(vis[sl].transpose(0, 2, 1)).astype(ct),
            "txtT": np.ascontiguousarray(txt[sl].transpose(0, 2, 1)).astype(ct),
            "wvp": wvp,
            "wtp": wtp,
            "bvp": bvp,
            "btp": btp,
            "ones": ones,
        })
    return in_maps


def run(inputs, trace=False, tmpdir=None, dtype="bf16"):
    """Returns (full_output, BassKernelResults)."""
    from concourse.bass_utils import run_bass_kernel_spmd

    nc = _get_nc(dtype)
    in_maps = _prep_in_maps(**inputs, dtype=dtype)
    res = run_bass_kernel_spmd(
        nc, in_maps, core_ids=list(range(NCORES)), trace=trace, tmpdir=tmpdir
    )
    outp = np.concatenate([res.results[c]["out"] for c in range(NCORES)], axis=0)
    return outp, res


def kernel(**inputs) -> np.ndarray:
    outp, _ = run(inputs, trace=False)
    return outp
